# revision 18
# baseline (speedup 1.0000x reference)
"""MultiHuberLoss Trainium2 kernel (v8: all-ap_gather extraction).

Reference (per element, with m = +x at the target class, -x elsewhere):
    hinge = max(0, 1 - m);  loss = where(m >= -1, hinge^2, -4m);  out = sum(loss)/N

Math (exact identities):
  Main pass treats every element as non-target (m = -x):
      F(-x) = (clamp(x,-1,1) + 1)^2 + 4*max(x, 1) - 4
  Per-row correction for the target column t: F(x_t) - F(-x_t) = -4 * x_t
  So per core:
      S = sum_ij (v+1)^2  +  4*sum_ij u  -  4*ROWS*C  -  4*sum_i x[i, t_i]
      with v = clamp(x,-1,1), u = max(x,1)

Engine split (per 2MB tile, all hidden under the ~80us DMA stream):
  - DVE:   v = clamp(x,-1,1) -> bf16 (tensor_scalar 2x mode)
           u = max(x,1)      -> bf16 (tensor_scalar 2x mode)
  - ACT:   Square(v + 1) with fused accum -> per-partition sums accA
  - PE:    column sums of u via ones^T @ u_chunk accumulated into one
           PSUM bank across all chunks/tiles (B term)
  - GPSIMD: target extraction via ap_gather ONLY (a ucode SBUF gather,
    ~0.4us/op, no SWDGE descriptor traffic -- indirect DMA descriptors
    measurably slow DMA engines 7/15 and lag every tile's completion).
    ap_gather shares its 16 indices across each 16-partition group, so:
      * j in [0,48): host pre-sorts rows by target class so all 16 rows
        of a (group, row-position) slot share one target -> one gather
        per tile covers its 4 row-positions (indices 0..3, pad 4..15).
      * j in [48,64): mixed rows; one gather per row-position with the
        16 rows' OWN target indices -> the diagonal out[16g+q, q] is
        row q's target element.
    A static 0/1 mask (1 at valid/diagonal slots) dot the gather block
    in ONE tensor_tensor_reduce yields the whole correction sum.

The row permutation is applied on host (the loss is a plain sum over
rows, so it is permutation invariant); bf16 intermediates are safely
inside the 2e-2 tolerance (measured rel err ~1e-7).

Scheduling notes (hard-won):
  - The tile scheduler reorders within engines; ops that consume gather
    results must be pinned to the end of the schedule or they stall the
    DVE mid-stream. The final reduces therefore write INTO the last u
    tile (fp32), whose WAW/WAR deps against the last tile's compute pin
    them after the whole pipeline.
  - 2MB tiles with a 5-deep x pool absorb per-tile completion jitter.
"""

import numpy as np

import concourse.bacc as bacc
import concourse.mybir as mybir
from concourse.bass_utils import run_bass_kernel_spmd
from concourse.tile import TileContext

N_TOTAL = 65536
C = 1000
N_CORES = 8
ROWS = N_TOTAL // N_CORES  # 8192 rows per core
P = 128                    # partitions
JPP = ROWS // P            # 64 row-positions per partition
FREE = JPP * C             # 64000 f32 per partition
NGRP = P // 16             # 8 gpsimd 16-partition groups
J_UNIF = 48                # row-positions extracted via shared-idx gathers
J_MIX = JPP - J_UNIF       # 16 row-positions via diagonal gathers
RPT = 4                    # row-positions per big tile (FD=4000)
NT_BIG = J_UNIF // RPT     # 12 shared-idx gather tiles
NIDX = NT_BIG + J_MIX      # 28 ap_gather ops / idx columns
TILE_FDS = [4000] * 15 + [2000] * 2
assert sum(TILE_FDS) == FREE

f32 = mybir.dt.float32
bf16 = mybir.dt.bfloat16
i16 = mybir.dt.int16
Alu = mybir.AluOpType
AF = mybir.ActivationFunctionType


def build_program():
    nc = bacc.Bacc(
        "TRN2", target_bir_lowering=False, debug=False, num_devices=N_CORES
    )
    x = nc.dram_tensor("x", [ROWS, C], f32, kind="ExternalInput")
    # ap_gather indices: cols 0..NT_BIG-1 shared-target (uniform slots),
    # cols NT_BIG.. per-row targets of mixed row-positions
    idx = nc.dram_tensor("idx", [P, NIDX], i16, kind="ExternalInput")
    # static 0/1 mask selecting valid/diagonal slots of the gather blocks
    msk = nc.dram_tensor("msk", [P, NIDX * 16], f32, kind="ExternalInput")
    out = nc.dram_tensor("out", [1, 1], f32, kind="ExternalOutput")

    x_flat = x.ap().rearrange("(p j) c -> p (j c)", p=P)  # [128, 64000]

    NT = len(TILE_FDS)
    jstarts = np.cumsum([0] + TILE_FDS[:-1]) // C

    with TileContext(nc) as tc:
        with (
            tc.tile_pool(name="xp", bufs=5) as xp,
            tc.tile_pool(name="vp", bufs=2) as vp,
            tc.tile_pool(name="up", bufs=2) as up,
            tc.tile_pool(name="scr", bufs=1) as scr,
            tc.tile_pool(name="small", bufs=1) as small,
            tc.tile_pool(name="psp", bufs=1, space="PSUM") as psp,
        ):
            # ACT-only discard output for the Square pass (one buffer,
            # written only by Scalar so no cross-engine WAW stalls)
            sq_scr = scr.tile([P, max(TILE_FDS)], bf16, tag="sq_scr")
            # x tile 0 queued first so the big stream leads the Sync queue
            xt0 = xp.tile([P, TILE_FDS[0]], f32)
            nc.sync.dma_start(out=xt0[:], in_=x_flat[:, 0:TILE_FDS[0]])

            idxs = small.tile([P, NIDX], i16, tag="idxs")
            nc.sync.dma_start(out=idxs[:], in_=idx.ap())
            mask = small.tile([P, NIDX * 16], f32, tag="mask")
            nc.sync.dma_start(out=mask[:], in_=msk.ap())
            ones_bf = small.tile([P, 1], bf16, tag="ones_bf")
            nc.vector.memset(ones_bf[:], 1.0)
            ones_f = small.tile([P, 1], f32, tag="ones_f")
            nc.vector.memset(ones_f[:], 1.0)

            accA = small.tile([P, NT], f32, tag="accA")
            gat = small.tile([P, NIDX * 16], f32, tag="gat")
            psB = psp.tile([1, 512], f32, tag="psB")

            # ---- main streaming loop ----
            off = 0
            n_chunks_total = sum(fd // 500 for fd in TILE_FDS)
            ci = 0
            u_lastf = None
            for t, fd in enumerate(TILE_FDS):
                if t == 0:
                    xt = xt0
                else:
                    xt = xp.tile([P, fd], f32)
                    nc.sync.dma_start(
                        out=xt[:], in_=x_flat[:, off:off + fd]
                    )
                v = vp.tile([P, fd], bf16)
                nc.vector.tensor_scalar(
                    v[:], xt[:], -1.0, 1.0, Alu.max, Alu.min
                )
                nc.scalar.activation(
                    sq_scr[:, 0:fd],
                    v[:],
                    AF.Square,
                    bias=1.0,
                    scale=1.0,
                    accum_out=accA[:, t:t + 1],
                )
                last = t == NT - 1
                u = up.tile([P, fd], f32 if last else bf16)
                if last:
                    u_lastf = u
                nc.vector.tensor_scalar(u[:], xt[:], 1.0, None, Alu.max)
                for c in range(fd // 500):
                    nc.tensor.matmul(
                        out=psB[:, 0:500],
                        lhsT=ones_f[:] if last else ones_bf[:],
                        rhs=u[:, c * 500:(c + 1) * 500],
                        start=(ci == 0),
                        stop=(ci == n_chunks_total - 1),
                    )
                    ci += 1
                # target extraction for this tile's row-positions
                jstart = int(jstarts[t])
                jend = jstart + fd // C
                if t < NT_BIG:
                    cols = [t]               # uniform: 4 positions, 1 op
                else:
                    cols = [NT_BIG + (j - J_UNIF)
                            for j in range(jstart, jend)]
                for col in cols:
                    nc.gpsimd.ap_gather(
                        out_ap=gat[:, col * 16:(col + 1) * 16],
                        in_ap=xt[:],
                        idxs_ap=idxs[:, col:col + 1],
                        channels=P, num_elems=fd, d=1, num_idxs=16,
                    )
                off += fd
            assert ci == n_chunks_total

            # ---- final combine ----
            # rG = sum(mask * gat) per partition in ONE op; outputs land in
            # the last (fp32) u tile so its WAW/WAR deps pin them after the
            # whole pipeline (a free-floating reduce gets scheduled mid-DVE
            # stream and stalls on the gather chain).
            W = NIDX * 16
            rG = u_lastf[:, W + 0:W + 1]
            nc.vector.tensor_tensor(
                u_lastf[:, 0:W], mask[:], gat[:], Alu.mult
            )
            nc.vector.reduce_sum(
                rG, u_lastf[:, 0:W], axis=mybir.AxisListType.X
            )
            rA = u_lastf[:, W + 1:W + 2]
            nc.vector.reduce_sum(rA, accA[:], axis=mybir.AxisListType.X)
            # u1 = rA - 4*rG  (per-partition)
            u1 = small.tile([P, 1], f32, tag="u1")
            nc.vector.scalar_tensor_tensor(
                out=u1[:], in0=rG, scalar=-4.0, in1=rA,
                op0=Alu.mult, op1=Alu.add,
            )
            psS = psp.tile([1, 8], f32, tag="psS")
            nc.tensor.matmul(
                out=psS[:, 0:1], lhsT=ones_f[:], rhs=u1[:],
                start=True, stop=True,
            )
            # sB = sum over the accumulated B bank
            sb_scr = small.tile([1, 500], f32, tag="sb_scr")
            sB = small.tile([1, 1], f32, tag="sB")
            nc.scalar.activation(
                sb_scr[:], psB[:, 0:500], AF.Identity,
                bias=0.0, scale=1.0, accum_out=sB[:],
            )
            # tmp = 4*sB + psS ;  res = tmp/N - 4*ROWS*C/N
            tmp = small.tile([1, 1], f32, tag="tmp")
            nc.vector.scalar_tensor_tensor(
                out=tmp[:], in0=sB[:], scalar=4.0, in1=psS[:, 0:1],
                op0=Alu.mult, op1=Alu.add,
            )
            biasc = -4.0 * ROWS * C / N_TOTAL  # = -500.0
            bias_t = small.tile([1, 1], f32, tag="bias")
            nc.vector.memset(bias_t[:], biasc)
            res = small.tile([1, 1], f32, tag="res")
            nc.scalar.activation(
                res[:], tmp[:], AF.Identity,
                bias=bias_t[:], scale=1.0 / N_TOTAL,
            )
            nc.sync.dma_start(out=out.ap(), in_=res[:])

    nc.compile()
    return nc


# ---------------- host-side placement ----------------

def build_placement(target):
    """Sort rows by target class and pack them so every 16-partition-group
    slot with j<J_UNIF is target-uniform. Returns (perm, idx16, mask)."""
    target = np.asarray(target).astype(np.int64)
    order = np.argsort(target, kind="stable")
    tsort = target[order]
    changes = np.flatnonzero(np.diff(tsort)) + 1
    starts = np.concatenate(([0], changes))
    ends = np.concatenate((changes, [N_TOTAL]))

    unif_list = []
    leftover_parts = []
    for s, e in zip(starts, ends):
        nfull = (e - s) // 16
        if nfull:
            unif_list.append(order[s:s + 16 * nfull].reshape(nfull, 16))
        if s + 16 * nfull < e:
            leftover_parts.append(order[s + 16 * nfull:e])
    unif = (
        np.concatenate(unif_list, axis=0)
        if unif_list else np.empty((0, 16), np.int64)
    )
    leftover = (
        np.concatenate(leftover_parts)
        if leftover_parts else np.empty(0, np.int64)
    )
    assert leftover.size % 16 == 0
    mixed = leftover.reshape(-1, 16)

    n_unif_needed = N_CORES * NGRP * J_UNIF  # 3072
    assert unif.shape[0] >= n_unif_needed, (
        f"not enough uniform 16-row groups: {unif.shape[0]} < {n_unif_needed}"
    )
    spill = unif[n_unif_needed:]
    unif = unif[:n_unif_needed]
    mix = np.concatenate([mixed, spill], axis=0)
    assert mix.shape[0] == N_CORES * NGRP * J_MIX  # 1024 exactly

    perm = np.empty(N_TOTAL, np.int64)
    q = np.arange(16)
    # uniform slots: chunk index ci -> (core, g, j) with j fastest
    ci = np.arange(n_unif_needed)
    core = ci // (NGRP * J_UNIF)
    rem = ci % (NGRP * J_UNIF)
    g = rem // J_UNIF
    j = rem % J_UNIF
    dest = (core * ROWS)[:, None] + (16 * g[:, None] + q[None, :]) * JPP \
        + j[:, None]
    perm[dest.ravel()] = unif.ravel()
    # mixed slots
    mi = np.arange(mix.shape[0])
    core = mi // (NGRP * J_MIX)
    rem = mi % (NGRP * J_MIX)
    g = rem // J_MIX
    j = J_UNIF + rem % J_MIX
    dest = (core * ROWS)[:, None] + (16 * g[:, None] + q[None, :]) * JPP \
        + j[:, None]
    perm[dest.ravel()] = mix.ravel()

    tgt_perm = target[perm].reshape(N_CORES, P, JPP)

    # per-tile j ranges
    jstarts = np.cumsum([0] + TILE_FDS[:-1]) // C

    idx16 = np.zeros((N_CORES, P, NIDX), np.int16)
    # uniform columns: idx16[core, 16g+m, t] = m*C + tgt(g, RPT*t+m), m<RPT
    m = np.arange(RPT)
    for gg in range(NGRP):
        tg = tgt_perm[:, 16 * gg, :J_UNIF].reshape(N_CORES, NT_BIG, RPT)
        vals = (m[None, None, :] * C + tg).astype(np.int16)
        idx16[:, 16 * gg + m, :NT_BIG] = vals.transpose(0, 2, 1)
    # mixed columns: per-row own target, tile-local row offset
    for j in range(J_UNIF, JPP):
        t = next(tt for tt in range(len(TILE_FDS))
                 if jstarts[tt] <= j < jstarts[tt] + TILE_FDS[tt] // C)
        jloc = j - int(jstarts[t])
        idx16[:, :, NT_BIG + (j - J_UNIF)] = (
            jloc * C + tgt_perm[:, :, j]
        ).astype(np.int16)

    # static mask: uniform cols -> k<RPT; mixed cols -> diagonal p%16==k
    mask = np.zeros((P, NIDX * 16), np.float32)
    k = np.arange(16)
    for t in range(NT_BIG):
        mask[:, 16 * t:16 * (t + 1)] = (k[None, :] < RPT)
    pmod = np.arange(P) % 16
    diag = (k[None, :] == pmod[:, None]).astype(np.float32)
    for c in range(NT_BIG, NIDX):
        mask[:, 16 * c:16 * (c + 1)] = diag
    return perm, idx16, mask


_NC_CACHE = None
LAST_RESULTS = None


def kernel(input, target):
    global _NC_CACHE, LAST_RESULTS
    x = np.ascontiguousarray(np.asarray(input, dtype=np.float32))
    tg = np.ascontiguousarray(np.asarray(target).astype(np.int64))
    assert x.shape == (N_TOTAL, C), x.shape
    assert tg.shape == (N_TOTAL,), tg.shape

    if _NC_CACHE is None:
        _NC_CACHE = build_program()
    nc = _NC_CACHE

    perm, idx16, mask = build_placement(tg)
    x_perm = np.ascontiguousarray(x[perm])

    in_maps = [
        {
            "x": x_perm[c * ROWS:(c + 1) * ROWS],
            "idx": idx16[c],
            "msk": mask,
        }
        for c in range(N_CORES)
    ]
    res = run_bass_kernel_spmd(nc, in_maps, core_ids=list(range(N_CORES)))
    LAST_RESULTS = res
    total = np.float32(0.0)
    for r in res.results:
        total += np.float32(r["out"].reshape(()))
    return np.asarray(total, dtype=np.float32)


if __name__ == "__main__":
    rng = np.random.default_rng(0)
    xs = rng.standard_normal((N_TOTAL, C), dtype=np.float32)
    ts = rng.integers(0, C, size=(N_TOTAL,)).astype(np.int64)
    got = kernel(xs, ts)
    m = np.where(np.arange(C)[None, :] == ts[:, None], xs, -xs)
    hinge = np.maximum(0.0, 1.0 - m)
    loss = np.where(m >= -1.0, hinge * hinge, -4.0 * m)
    want = loss.sum(dtype=np.float64) / N_TOTAL
    print("got", got, "want", want, "rel", abs(got - want) / abs(want))
